# revision 3
# baseline (speedup 1.0000x reference)
"""Trainium2 Bass kernel for an attention-style graph convolution (GAT layer).

v7 = v3 (4x2 sharding, ACT relu-split offload) with progressive tiling:
the first j-chunks are processed as small units (1,1,2 chunks) so the
DMA->ts->tt->matmul pipeline fills fast, then steady-state runs on 4-chunk
units (low per-op overhead). Drain uses vector-only copies into one batched
tile and 4 grouped output DMAs on both rings.

Math (mask stream pre-scaled: mt[j,i] = m[i,j]*es2a_j, fp16):
    n[j,i] = mt * max(es1b_i*es2m_j, 1) = mt * (1 + relu(es1b_i*es2m_j - 1))
DVE-units: u = max(es1b*es2m_j, 1) (ts) ; n = u*mt (tt) ; 16 mm/chunk
ACT-units: w = relu(es1b*es2m_j - 1) (ACT engine) ; n1 = w*mt (tt)
           32 mm/chunk (n1.T@g2 plus mt.T@g2 -- the "+1" chain on the PE)
j-half partials summed across core pairs on the host, then divide + elu.
"""

import ml_dtypes
import numpy as np

import concourse.bacc as bacc
import concourse.bass as bass
import concourse.mybir as mybir
import concourse.tile as tile
from concourse import bass_utils

F32 = mybir.dt.float32
FP16 = mybir.dt.float16
AF = mybir.ActivationFunctionType
OP = mybir.AluOpType

N = 8192
K = 256
F = 128
ALPHA = 0.2
NCORES = 8
MI = 2048         # i-columns per core
MJ = 4096         # j-rows per core
P = 128
NJ = MJ // P      # 32 j-chunks
NIT = MI // P     # 16 i-blocks

UNIT_SIZES = [1, 1, 2, 4, 4, 4, 4, 4, 4, 4]   # chunks per unit (sum 32)
ACT_UNITS = frozenset({4, 6, 8})              # units built on the ACT engine
LAG = 2                                        # pipeline depth in units


def _broadcast_ap(row_ap, nparts):
    return bass.AP(
        tensor=row_ap.tensor,
        offset=row_ap.offset,
        ap=[[0, nparts]] + [list(d) for d in row_ap.ap],
    )


def build_program():
    nc = bacc.Bacc("TRN2", target_bir_lowering=False)

    mt_d = nc.dram_tensor("mt", (MJ, MI), FP16, kind="ExternalInput")
    g2_d = nc.dram_tensor("g2", (P, NJ * (F + 1)), FP16, kind="ExternalInput")
    es1b_d = nc.dram_tensor("es1b", (1, MI), FP16, kind="ExternalInput")
    es2m_d = nc.dram_tensor("es2m", (P, NJ), F32, kind="ExternalInput")
    out_d = nc.dram_tensor("out", (MI, F + 1), F32, kind="ExternalOutput")

    with tile.TileContext(nc) as tc:
        with (
            tc.tile_pool(name="consts", bufs=1) as consts,
            tc.tile_pool(name="adjp", bufs=4) as adjp,
            tc.tile_pool(name="up", bufs=2) as up,
            tc.tile_pool(name="ntp", bufs=3) as ntp,
            tc.tile_pool(name="gsp", bufs=2) as gsp,
            tc.tile_pool(name="outp", bufs=1) as outp,
            tc.tile_pool(name="ps_acc", bufs=1, space="PSUM") as ps_acc,
        ):
            es2m = consts.tile([P, NJ], F32, tag="es2m")
            es1b = consts.tile([P, MI], FP16, tag="es1b")
            neg1 = consts.tile([P, 1], F32, tag="neg1")
            nc.gpsimd.memset(neg1[:], -1.0)
            # es-vectors on sync; the first mask units go on scalar so the
            # latency-critical chunk-0 transfer is not queued behind es1b
            nc.sync.dma_start(out=es2m[:], in_=es2m_d[:, :])
            nc.sync.dma_start(out=es1b[:], in_=_broadcast_ap(es1b_d[:, :], P))

            accs = [
                ps_acc.tile([P, 512], F32, tag=f"acc{b}", name=f"acc{b}")
                for b in range(8)
            ]

            def acc_slice(it):
                return accs[it // 2][:, (it % 2) * 256 : (it % 2) * 256 + F + 1]

            mt_r = mt_d.rearrange("(c p) m -> p c m", p=P)
            unit_off = np.cumsum([0] + UNIT_SIZES).tolist()

            pend = []
            gs_slab = [None]

            def phase_a(un):
                off, sz = unit_off[un], UNIT_SIZES[un]
                if off % 8 == 0:
                    g8 = off // 8
                    gs = gsp.tile([P, 8 * (F + 1)], FP16, tag="gs")
                    nc.sync.dma_start(
                        out=gs[:],
                        in_=g2_d[:, g8 * 8 * (F + 1) : (g8 + 1) * 8 * (F + 1)],
                    )
                    gs_slab[0] = gs
                adj_t = adjp.tile([P, 4, MI], FP16, tag="adj")
                eng = nc.scalar if un % 2 == 0 else nc.sync
                eng.dma_start(
                    out=adj_t[:, :sz, :], in_=mt_r[:, off : off + sz, :]
                )
                pend.append((un, adj_t, gs_slab[0]))

            def phase_c():
                un, adj_t, gs = pend.pop(0)
                off, sz = unit_off[un], UNIT_SIZES[un]
                is_act = un in ACT_UNITS
                u_t = up.tile([P, 4, MI], FP16, tag="u_t")
                for q in range(sz):
                    jc = off + q
                    if is_act:
                        nc.scalar.activation(
                            u_t[:, q, :], es1b[:], AF.Relu,
                            bias=neg1[:], scale=es2m[:, jc : jc + 1],
                        )
                    else:
                        nc.vector.tensor_scalar(
                            out=u_t[:, q, :],
                            in0=es1b[:],
                            scalar1=es2m[:, jc : jc + 1],
                            scalar2=1.0,
                            op0=OP.mult,
                            op1=OP.max,
                        )
                n_t = ntp.tile([P, 4, MI], FP16, tag="n_t")
                nc.vector.tensor_tensor(
                    out=n_t[:, :sz, :], in0=u_t[:, :sz, :],
                    in1=adj_t[:, :sz, :], op=OP.mult,
                )
                for q in range(sz):
                    jc = off + q
                    gsl = gs[:, (jc % 8) * (F + 1) : (jc % 8) * (F + 1) + F + 1]
                    for it in range(NIT):
                        nc.tensor.matmul(
                            acc_slice(it),
                            n_t[:, q, it * P : (it + 1) * P],
                            gsl,
                            start=(jc == 0 and it % 2 == 0),
                            stop=(jc == NJ - 1),
                            skip_group_check=True,
                        )
                        if is_act:
                            nc.tensor.matmul(
                                acc_slice(it),
                                adj_t[:, q, it * P : (it + 1) * P],
                                gsl,
                                start=False,
                                stop=False,
                                skip_group_check=True,
                            )

            nunits = len(UNIT_SIZES)
            for un in range(nunits):
                phase_a(un)
                if un >= LAG:
                    phase_c()
            while pend:
                phase_c()

            # drain: vector-only copies into one batched tile, grouped DMAs
            out_r = out_d.rearrange("(c p) f -> p c f", p=P)
            res = outp.tile([P, NIT, F + 1], F32, tag="res")
            for g in range(4):
                for k in range(4):
                    it = 4 * g + k
                    nc.vector.tensor_copy(res[:, it, :], acc_slice(it))
                eng = nc.sync if g % 2 == 0 else nc.scalar
                eng.dma_start(
                    out=out_r[:, 4 * g : 4 * g + 4, :],
                    in_=res[:, 4 * g : 4 * g + 4, :],
                )

    nc.compile()
    return nc


_NC_CACHE = [None]


def _get_nc():
    if _NC_CACHE[0] is None:
        _NC_CACHE[0] = build_program()
    return _NC_CACHE[0]


def host_prepare(x, adj, W, a):
    h64 = x.astype(np.float64) @ W.astype(np.float64)
    s1 = h64 @ a[:F, 0].astype(np.float64)
    s2 = h64 @ a[F:, 0].astype(np.float64)
    es2a = np.exp(ALPHA * s2)
    es2m = np.exp((1.0 - ALPHA) * s2)
    g2 = np.empty((N, F + 1), np.float64)
    g2[:, :F] = h64
    g2[:, F] = 1.0
    g2 = g2.astype(np.float16)
    es1b = np.exp((1.0 - ALPHA) * s1).astype(np.float16)
    maskT = adj.T > 0
    mt_full = np.where(maskT, es2a[:, None], 0.0).astype(np.float16)

    in_maps = []
    for c in range(NCORES):
        si = c % 4
        hj = c // 4
        isl = slice(si * MI, (si + 1) * MI)
        jsl = slice(hj * MJ, (hj + 1) * MJ)
        g2h = np.ascontiguousarray(
            g2[jsl].reshape(NJ, P, F + 1).transpose(1, 0, 2).reshape(P, NJ * (F + 1))
        )
        es2mh = np.ascontiguousarray(es2m[jsl].reshape(NJ, P).T.astype(np.float32))
        in_maps.append(
            {
                "mt": np.ascontiguousarray(mt_full[jsl, isl]),
                "g2": g2h,
                "es1b": es1b[isl].reshape(1, MI),
                "es2m": es2mh,
            }
        )
    return in_maps


def kernel(x, adj, W, a, _trace=False):
    x = np.asarray(x)
    adj = np.asarray(adj)
    W = np.asarray(W)
    a = np.asarray(a)

    in_maps = host_prepare(x, adj, W, a)
    nc = _get_nc()
    res = bass_utils.run_bass_kernel_spmd(
        nc, in_maps, core_ids=list(range(NCORES)), trace=_trace
    )
    slabs = []
    for si in range(4):
        slabs.append(res.results[si]["out"] + res.results[si + 4]["out"])
    nd = np.concatenate(slabs, axis=0)
    hp = nd[:, :F] / nd[:, F : F + 1]
    out = np.where(hp > 0, hp, np.expm1(np.minimum(hp, 0.0))).astype(np.float32)
    if _trace:
        return out, res
    return out
